# revision 37
# baseline (speedup 1.0000x reference)
"""Trainium2 Bass kernel for nn_NodeInference (2-layer GAT + cosine head).

v2 design (SPMD over 8 cores, dst-node sharding):
  Host globally re-assigns nodes to (core, block) bins, balancing per-block
  in-degree so chunk capacities carry minimal padding. Self-loops are NOT in
  the edge lists; each block handles them as one extra "virtual chunk" fed
  directly from the block's own 128-row window (no gather).

  Packed row tables in HBM, 384 fp16 cols (768B, dma_gather-compatible):
     L1 row = [h fp16 x256 | a_src f32 x2 | a_dst f32 x2 | pad]
     L2 row = [h fp16 x256 | a_src f32 | a_dst f32 | pad]
  P1  dense-1 (replicated, permuted order): h1aug = x @ W1aug -> h1 table
  P2  edge phase (per dst block of 128 nodes):
      - direct window load of the block's own rows (a_dst source + self loop)
      - bulk src-row gather via dma_gather (int16 idx; lo/hi table halves)
      - PE broadcast of per-chunk dst-locals (ones[1,128]^T @ dstrow) ->
        S_T[d,e] = is_equal(bcast, iota_col); a_d per edge = S_T^T @ adwin
      - w_e = exp(min(leakyrelu(a_s + a_d, 0.2), 30))
      - Sw_h[e,d] = (iota == dstloc) * w_h  (fused DVE op); PSUM accum:
        bp[:,hP:(h+1)P] += Sw_h^T @ h_rows ; bp[:,2P+h] += Sw_h^T @ ones
      - out1 = leakyrelu(bp_h/SumW_h + b1, 0.01); PE-transpose -> out1T
      - h2aug = out1 @ W2aug -> packed rows -> cc_in (local shard rows)
  P3  AllGather cc_in -> cc_out (row index == global node position)
  P4  edge phase 2 (1 head) over cc_out/cc_in
  P5  head: cos sim vs mu -> outT [8, SHARD_CAP]
Host scatters per-core outT into the full output via the assignment map.
"""

import sys
from dataclasses import dataclass, field
from contextlib import ExitStack

if "/opt/trn_rl_repo" not in sys.path:
    sys.path.insert(0, "/opt/trn_rl_repo")

import numpy as np

import concourse.bacc as bacc
import concourse.bass as bass
import concourse.mybir as mybir
import concourse.tile as tile
from concourse.bass import AP

P = 128
IN = 256          # input feature dim
H1 = 2            # layer-1 heads
HID = 256         # layer-1 output dim (2*128, concat)
OUT = 256         # layer-2 output dim
KH, MD = 8, 128   # cosine head shape
ROWW = 384        # fp16 cols per packed table row (768B)
HALF = 32768      # int16 table-half split
AF = mybir.ActivationFunctionType
ALU = mybir.AluOpType
DT = mybir.dt
USE_COLLECTIVE = True


@dataclass
class CFG:
    N: int
    W: int              # world size
    NBLK: int           # dst blocks (128 dsts) per core
    CPL1: int           # lo-half chunks per block, layer 1
    CPH1: int
    CPL2: int
    CPH2: int
    NT: int             # dense-1 node tiles
    idxmaps: object = field(default=None, repr=False)

    @property
    def Npad(self):
        return self.NT * P

    @property
    def SHARD_CAP(self):
        return self.NBLK * P

    @property
    def CPB1(self):
        return self.CPL1 + self.CPH1

    @property
    def CPB2(self):
        return self.CPL2 + self.CPH2


def build_program(cfg: CFG):
    nc = bacc.Bacc("TRN2", target_bir_lowering=False, debug=False)
    W, NBLK = cfg.W, cfg.NBLK
    AUG1, AUG2 = IN + 4, IN + 2
    f16, bf16, f32 = DT.float16, DT.bfloat16, DT.float32
    i16 = DT.int16

    with tile.TileContext(nc) as tc, ExitStack() as stack:
        dram = stack.enter_context(
            tc.tile_pool(name="dram", bufs=1, space="DRAM"))

        def din(name, shape, dtype):
            return dram.tile(shape, dtype, kind="ExternalInput", name=name,
                             uniquify=False)

        xTi = din("xTi", [P, cfg.NT, 2, P], f16)
        w1s = din("w1s", [P, 2, AUG1], f16)
        w2s = din("w2s", [P, 2, AUG2], f16)
        gsd = din("gs", [P, 2, KH * P], f16)
        mus = din("mus", [P, KH * KH], f16)       # block-diag mu^T
        ond = din("onesd", [P, KH * KH], f16)     # block-diag ones
        cmu = din("cmu", [KH, 1], f32)
        b1d = din("b1b", [P, HID], f32)
        b2d = din("b2b", [P, OUT], f32)
        iot = din("iota", [P, P], bf16)
        ioc = din("iotac", [P, 1], f32)           # iota column (partition idx)
        one1 = din("ones1", [1, P], bf16)         # bcast matmul lhsT
        idn = din("ident", [P, P], f32)
        is1 = din("isd1", [P, NBLK * cfg.CPB1 * 9], i16)
        dr1 = din("dstrow1", [1, NBLK * cfg.CPB1 * P], bf16)
        is2 = din("isd2", [P, NBLK * cfg.CPB2 * 9], i16)
        dr2 = din("dstrow2", [1, NBLK * cfg.CPB2 * P], bf16)
        outT = dram.tile([KH, cfg.SHARD_CAP], f32, kind="ExternalOutput",
                         name="outT", uniquify=False)

        h1lo = dram.tile([min(HALF, cfg.Npad), ROWW], f16,
                         name="h1_lo")
        h1hi = dram.tile([max(cfg.Npad - HALF, P), ROWW], f16,
                         name="h1_hi")
        cc_in = dram.tile([cfg.SHARD_CAP, ROWW], f16, name="cc_in")
        cc_out = dram.tile([W * cfg.SHARD_CAP, ROWW], f16, name="cc_out",
                           addr_space="Shared" if W > 1 else "Local")

        consts = stack.enter_context(tc.tile_pool(name="consts", bufs=1))
        w1_sb = consts.tile([P, 2, AUG1], f16)
        w2_sb = consts.tile([P, 2, AUG2], f16)
        g_sb = consts.tile([P, 2, KH * P], f16)
        mu_sb = consts.tile([P, KH * KH], f16)
        on_sb = consts.tile([P, KH * KH], f16)
        cmu_sb = consts.tile([KH, 1], f32)
        b1_sb = consts.tile([P, HID], f32)
        b2_sb = consts.tile([P, OUT], f32)
        iota_sb = consts.tile([P, P], bf16)
        iotac_sb = consts.tile([P, 1], f32)
        ones1_sb = consts.tile([1, P], bf16)
        ident_sb = consts.tile([P, P], f32)
        out1T_sb = consts.tile([P, 2, cfg.SHARD_CAP], f16)
        h2fT_sb = consts.tile([P, 2, cfg.SHARD_CAP], f16)

        for dst, src in [(w1_sb, w1s), (w2_sb, w2s), (g_sb, gsd),
                         (mu_sb, mus), (on_sb, ond), (cmu_sb, cmu),
                         (b1_sb, b1d), (b2_sb, b2d), (iota_sb, iot),
                         (iotac_sb, ioc), (ones1_sb, one1),
                         (ident_sb, idn)]:
            nc.sync.dma_start(dst[:], src[:])

        if cfg.Npad <= HALF:
            # h1hi is a dummy (P1 never writes it); zero it so padding
            # gathers read finite values.
            zhi = consts.tile([P, ROWW], f16)
            nc.vector.memset(zhi[:], 0.0)
            nc.sync.dma_start(h1hi[0:P, :], zhi[:])

        # ================= P1: dense layer 1 (replicated, permuted) =========
        # 8 tiles per DMA batch: HWDGE dispatch (~0.6us/dma) dominates at
        # one-tile granularity.
        TB = 8
        with tc.tile_pool(name="p1x", bufs=3) as p1x, \
             tc.tile_pool(name="p1ps", bufs=4, space="PSUM") as p1ps, \
             tc.tile_pool(name="p1row", bufs=3) as p1row:
            for t0 in range(0, cfg.NT, TB):
                tb = min(TB, cfg.NT - t0)
                xt = p1x.tile([P, TB, 2, P], f16, tag="xt")
                nc.sync.dma_start(xt[:, 0:tb, :, :], xTi[:, t0:t0 + tb, :, :])
                # row layout: [h1 0:128 | 1.0 | h2 129:257 | 1.0 |
                #              a_s f32 @129:131 | a_d f32 @131:133]
                row = p1row.tile([P, TB, ROWW], f16, tag="row")
                rf32 = row[:].bitcast(f32)
                for i in range(tb):
                    ps = p1ps.tile([P, AUG1], f32, tag="ps")
                    for k in range(2):
                        nc.tensor.matmul(ps[:], lhsT=xt[:, i, k, :],
                                         rhs=w1_sb[:, k, :],
                                         start=(k == 0), stop=(k == 1))
                    # split the PSUM->row copies across ACT and DVE: both
                    # engines are otherwise idle in P1 and ACT alone was the
                    # phase critical path.
                    nc.scalar.activation(row[:, i, 0:P], ps[:, 0:P], AF.Copy)
                    nc.vector.tensor_copy(row[:, i, P + 1:IN + 1],
                                          ps[:, P:IN])
                    nc.vector.tensor_copy(rf32[:, i, 129:133],
                                          ps[:, IN:IN + 4])
                nc.vector.memset(row[:, 0:tb, P:P + 1], 1.0)
                nc.vector.memset(row[:, 0:tb, IN + 1:IN + 2], 1.0)
                # tiles t<HALF//P land in h1lo, the rest in h1hi (TB
                # divides HALF//P, so a batch never straddles)
                if t0 * P < HALF:
                    hap, off = h1lo[:], t0 * P * ROWW
                else:
                    hap, off = h1hi[:], (t0 * P - HALF) * ROWW
                dst = AP(hap.tensor, off,
                         [[ROWW, P], [P * ROWW, tb], [1, IN + 10]])
                nc.sync.dma_start(dst, row[:, 0:tb, 0:IN + 10])

        # ================= P2/P4: edge phases ================================
        def edge_phase(layer, post_block=None):
            if layer == 1:
                tab_lo, tab_hi = h1lo[:, :], h1hi[:, :]

                def dst_win(blk):
                    return h1lo[blk * P:(blk + 1) * P, :]
                isrc_d, drow_d = is1, dr1
                CPL, CPH, CPB = cfg.CPL1, cfg.CPH1, cfg.CPB1
                nhead = 2
                b_sb, out_t, lrelu_out = b1_sb, out1T_sb, True
                as_off, ad_off = 129, 131   # f32 col offsets
            else:
                ccrows = W * cfg.SHARD_CAP
                tab_lo = cc_out[0:min(HALF, ccrows), :]
                tab_hi = (cc_out[HALF:ccrows, :] if ccrows > HALF
                          else tab_lo)

                def dst_win(blk):
                    return cc_in[blk * P:(blk + 1) * P, :]
                isrc_d, drow_d = is2, dr2
                CPL, CPH, CPB = cfg.CPL2, cfg.CPH2, cfg.CPB2
                nhead = 1
                b_sb, out_t, lrelu_out = b2_sb, h2fT_sb, False
                as_off, ad_off = 129, 130

            estack = ExitStack()
            pi = estack.enter_context(
                tc.tile_pool(name=f"idx{layer}", bufs=4))
            pg = estack.enter_context(
                tc.tile_pool(name=f"gath{layer}", bufs=4))
            pw = estack.enter_context(
                tc.tile_pool(name=f"win{layer}", bufs=3))
            pb = estack.enter_context(
                tc.tile_pool(name=f"bcps{layer}", bufs=1, space="PSUM"))
            pst = estack.enter_context(tc.tile_pool(name=f"st{layer}", bufs=3))
            pad_ = estack.enter_context(
                tc.tile_pool(name=f"adps{layer}", bufs=1, space="PSUM"))
            pe_ = estack.enter_context(tc.tile_pool(name=f"ew{layer}", bufs=2))
            pm = estack.enter_context(tc.tile_pool(name=f"sw{layer}", bufs=2))
            pp = estack.enter_context(
                tc.tile_pool(name=f"bps{layer}", bufs=2, space="PSUM"))
            pt = estack.enter_context(
                tc.tile_pool(name=f"tps{layer}", bufs=1, space="PSUM"))
            po = estack.enter_context(tc.tile_pool(name=f"epi{layer}", bufs=2))
            ph = estack.enter_context(
                tc.tile_pool(name=f"h2ps{layer}", bufs=1, space="PSUM"))

            NSW = CPB + 1   # chunks + self-loop virtual chunk

            for blk in range(NBLK):
                # --- per-block inputs (isrc idx + dstf packed in one DMA)
                cb9 = blk * CPB * 9
                isd = pi.tile([P, CPB * 9], i16, tag="isd")
                drow = pi.tile([1, CPB * P], bf16, tag="drow")
                nc.sync.dma_start(isd[:], isrc_d[:, cb9:cb9 + CPB * 9])
                nc.sync.dma_start(drow[:], drow_d[:, blk * CPB * P:
                                                  (blk + 1) * CPB * P])
                isrc = isd[:, 0:CPB * 8]
                dstf = isd[:, CPB * 8:CPB * 9].bitcast(bf16)
                win = pw.tile([P, ROWW], f16, tag="win")
                nc.sync.dma_start(win[:], dst_win(blk))
                winf = win[:].bitcast(f32)

                # --- src-row gathers (dma_gather caps at 1024 idxs = 8 chunks)
                gt = pg.tile([P, CPB, ROWW], f16, tag="gt")
                MXC = 8
                for c0 in range(0, CPL, MXC):
                    c1 = min(c0 + MXC, CPL)
                    nc.gpsimd.dma_gather(
                        gt[:, c0:c1, :], tab_lo,
                        isrc[:, c0 * 8:c1 * 8],
                        (c1 - c0) * P, (c1 - c0) * P, ROWW)
                for c0 in range(CPL, CPB, MXC):
                    c1 = min(c0 + MXC, CPB)
                    nc.gpsimd.dma_gather(
                        gt[:, c0:c1, :], tab_hi,
                        isrc[:, c0 * 8:c1 * 8],
                        (c1 - c0) * P, (c1 - c0) * P, ROWW)
                gtf = gt[:].bitcast(f32)

                # --- dst-local broadcast (PE) -> dstb (ACT copy) -> S_T via
                # one batched DVE is_equal over all chunks
                dstb = pst.tile([P, CPB * P], bf16, tag="dstb")
                st = pst.tile([P, CPB * P], bf16, tag="st")
                GW = 4  # chunks per broadcast matmul (512 psum cols)
                for g0 in range(0, CPB, GW):
                    g1 = min(g0 + GW, CPB)
                    bc = pb.tile([P, GW * P], f32, tag="bc")
                    nc.tensor.matmul(bc[:, 0:(g1 - g0) * P],
                                     lhsT=ones1_sb[:],
                                     rhs=drow[:, g0 * P:g1 * P],
                                     start=True, stop=True)
                    nc.scalar.activation(dstb[:, g0 * P:g1 * P],
                                         bc[:, 0:(g1 - g0) * P], AF.Copy)
                nc.vector.tensor_scalar(
                    out=st[:], in0=dstb[:], scalar1=iotac_sb[:, 0:1],
                    scalar2=None, op0=ALU.is_equal)

                # --- a_d per edge: ad_ps[e, (j,h)] = S_T_j^T @ adwin
                adw16 = pe_.tile([P, nhead], f16, tag="adw16")
                nc.vector.tensor_copy(adw16[:], winf[:, ad_off:ad_off + nhead])
                # one accumulation group for the whole tile: start=True zeroes
                # the full 2KB psum region, so only the first matmul may start
                # and only the last may stop; disjoint writes in between are
                # first-touch overwrites.
                ad_ps = pad_.tile([P, CPB * nhead], f32, tag="adps")
                for j in range(CPB):
                    nc.tensor.matmul(ad_ps[:, j * nhead:(j + 1) * nhead],
                                     lhsT=st[:, j * P:(j + 1) * P],
                                     rhs=adw16[:], start=(j == 0),
                                     stop=(j == CPB - 1))

                # --- edge weights w = exp(lrelu(a_s + a_d, .2))
                ew = pe_.tile([P, CPB * nhead], f32, tag="ew")
                wv = pe_.tile([P, CPB * nhead], bf16, tag="wv")
                as_v = gtf[:, :, as_off:as_off + nhead]
                nc.vector.tensor_tensor(
                    ew[:], as_v, ad_ps[:], op=ALU.add)
                ewl = pe_.tile([P, CPB * nhead], f32, tag="ewl")
                nc.vector.tensor_scalar(out=ewl[:], in0=ew[:], scalar1=0.2,
                                        scalar2=None, op0=ALU.mult)
                nc.vector.tensor_tensor(ewl[:], ewl[:], ew[:], op=ALU.max)
                nc.scalar.activation(wv[:], ewl[:], AF.Exp)

                # --- self-loop weights from the window rows
                ws = pe_.tile([P, nhead], f32, tag="ws")
                nc.vector.tensor_tensor(
                    ws[:], winf[:, as_off:as_off + nhead],
                    winf[:, ad_off:ad_off + nhead], op=ALU.add)
                wt = pe_.tile([P, nhead], f32, tag="wt")
                nc.vector.tensor_scalar(out=wt[:], in0=ws[:], scalar1=0.2,
                                        scalar2=None, op0=ALU.mult)
                nc.vector.tensor_tensor(wt[:], wt[:], ws[:], op=ALU.max)
                nc.scalar.activation(wt[:], wt[:], AF.Exp)

                # --- batched one-hot scatter matrices:
                #     s_all[e,(j,d)] = (dstf[e,j] == iota[d])
                #     sw_h[e,(j,d)]  = s_all * w_h[e,j]   (stride-0 bcasts)
                sall = pm.tile([P, CPB * P], bf16, tag="sall")
                dfa = dstf[:, 0:1]
                dview = AP(dfa.tensor, dfa.offset,
                           [dfa.ap[0], [1, CPB], [0, P]])
                ioa = iota_sb[:]
                iview = AP(ioa.tensor, ioa.offset,
                           [ioa.ap[0], [0, CPB], [1, P]])
                nc.vector.tensor_tensor(sall[:], dview, iview,
                                        op=ALU.is_equal)
                swh = []
                for h in range(nhead):
                    wvh = wv[:, h:h + 1]
                    wview = AP(wvh.tensor, wvh.offset,
                               [wvh.ap[0], [nhead, CPB], [0, P]])
                    swt = pm.tile([P, CPB * P], bf16, tag=f"swh{h}")
                    nc.vector.tensor_tensor(swt[:], sall[:], wview,
                                            op=ALU.mult)
                    swh.append(swt)

                # --- weighted one-hot scatter accumulation
                # single psum accumulation group across all j/h matmuls (see
                # zero-region note above): start only on the first matmul,
                # stop only on the very last.  The table rows carry a literal
                # 1.0 after each head's features, so one matmul accumulates
                # both the weighted feature sum and the softmax denominator:
                # bp layout [f1 0:128 | sumw1 | f2 129:257 | sumw2] (2 heads)
                # or [f 0:256 | sumw] (1 head).
                HB = P + 1 if nhead == 2 else IN + 1
                bp = pp.tile([P, nhead * HB], f32, tag="bp")
                for j in range(NSW):
                    last = (j == NSW - 1)
                    selfc = (j == CPB)
                    for h in range(nhead):
                        if selfc:
                            sw = pm.tile([P, P], bf16, tag="sw")
                            nc.vector.tensor_scalar(
                                out=sw[:], in0=iota_sb[:],
                                scalar1=iotac_sb[:, 0:1],
                                scalar2=wt[:, h:h + 1],
                                op0=ALU.is_equal, op1=ALU.mult)
                            lhsT = sw[:]
                        else:
                            lhsT = swh[h][:, j * P:(j + 1) * P]
                        c0, c1 = h * HB, (h + 1) * HB
                        rhs = win[:, c0:c1] if selfc else gt[:, j, c0:c1]
                        nc.tensor.matmul(bp[:, c0:c1], lhsT=lhsT,
                                         rhs=rhs,
                                         start=(j == 0 and h == 0),
                                         stop=(last and h == nhead - 1))

                # ---- block epilogue
                rec = po.tile([P, nhead], f32, tag="rec")
                for h in range(nhead):
                    nc.vector.reciprocal(rec[:, h:h + 1],
                                         bp[:, (h + 1) * HB - 1:
                                            (h + 1) * HB])
                ti = po.tile([P, IN], f32, tag="ti")
                if nhead == 2:
                    nc.scalar.activation(ti[:, 0:P], bp[:, 0:P], AF.Copy,
                                         scale=rec[:, 0:1])
                    nc.scalar.activation(ti[:, P:IN], bp[:, HB:HB + P],
                                         AF.Copy, scale=rec[:, 1:2])
                else:
                    nc.scalar.activation(ti[:], bp[:, 0:IN], AF.Copy,
                                         scale=rec[:, 0:1])
                nc.vector.tensor_tensor(ti[:], ti[:], b_sb[:], op=ALU.add)
                if lrelu_out:
                    # lrelu(x) = 0.01x + relu(0.99x)
                    tr = po.tile([P, IN], f32, tag="tr")
                    nc.scalar.activation(tr[:], ti[:], AF.Relu, scale=0.99)
                    nc.vector.scalar_tensor_tensor(
                        out=ti[:], in0=ti[:], scalar=0.01, in1=tr[:],
                        op0=ALU.mult, op1=ALU.add)
                for k in range(2):
                    tp = pt.tile([P, P], f32, tag="tp")
                    nc.tensor.transpose(tp[:], ti[:, k * P:(k + 1) * P],
                                        ident_sb[:])
                    nc.scalar.activation(out_t[:, k, blk * P:(blk + 1) * P],
                                         tp[:], AF.Copy)

                if layer == 1:
                    hp = ph.tile([P, AUG2], f32, tag="hp")
                    for k in range(2):
                        nc.tensor.matmul(
                            hp[:],
                            lhsT=out1T_sb[:, k, blk * P:(blk + 1) * P],
                            rhs=w2_sb[:, k, :], start=(k == 0), stop=(k == 1))
                    # L2 row: [h 0:256 | 1.0 | a_s f32 @129 | a_d f32 @130]
                    row2 = po.tile([P, ROWW], f16, tag="row2")
                    nc.scalar.activation(row2[:, 0:OUT], hp[:, 0:OUT], AF.Copy)
                    nc.vector.memset(row2[:, OUT:OUT + 2], 1.0)
                    r2f = row2[:].bitcast(f32)
                    nc.vector.tensor_copy(r2f[:, 129:131],
                                          hp[:, OUT:OUT + 2])
                    nc.sync.dma_start(
                        cc_in[blk * P:(blk + 1) * P, 0:OUT + 6],
                        row2[:, 0:OUT + 6])

                if post_block is not None:
                    post_block(blk)

            estack.close()

        edge_phase(1)

        nc.gpsimd.collective_compute(
            "AllGather", ALU.bypass,
            replica_groups=[list(range(W))],
            ins=[cc_in[:]], outs=[cc_out[:]])

        # ====== P5 head, interleaved into edge phase 2 ======================
        hstack = ExitStack()
        hps = hstack.enter_context(
            tc.tile_pool(name="hps", bufs=1, space="PSUM"))
        hsb = hstack.enter_context(tc.tile_pool(name="hsb", bufs=2))
        sps = hstack.enter_context(
            tc.tile_pool(name="sps", bufs=1, space="PSUM"))
        hepi = hstack.enter_context(tc.tile_pool(name="hepi", bufs=2))

        NTL = []
        _st = 0
        while _st < cfg.SHARD_CAP:
            _w = min(512, cfg.SHARD_CAP - _st)
            NTL.append((_st, _w))
            _st += _w

        def head_slice(st, wdt):
            nump = sps.tile([KH, 512], f32, tag="nump")
            nrmp = sps.tile([KH, 512], f32, tag="nrmp")
            for k in range(KH):
                hp = hps.tile([P, 512], f32, tag="hp")
                for f in range(2):
                    nc.tensor.matmul(hp[:, 0:wdt],
                                     lhsT=g_sb[:, f, k * P:(k + 1) * P],
                                     rhs=h2fT_sb[:, f, st:st + wdt],
                                     start=(f == 0), stop=(f == 1))
                h16 = hsb.tile([P, 512], f16, tag="h16")
                sq16 = hsb.tile([P, 512], f16, tag="sq16")
                nc.vector.tensor_copy(h16[:, 0:wdt], hp[:, 0:wdt])
                nc.scalar.activation(sq16[:, 0:wdt], hp[:, 0:wdt], AF.Square)
                nc.tensor.matmul(nump[:, 0:wdt],
                                 lhsT=mu_sb[:, k * KH:(k + 1) * KH],
                                 rhs=h16[:, 0:wdt], start=(k == 0),
                                 stop=(k == KH - 1))
                nc.tensor.matmul(nrmp[:, 0:wdt],
                                 lhsT=on_sb[:, k * KH:(k + 1) * KH],
                                 rhs=sq16[:, 0:wdt], start=(k == 0),
                                 stop=(k == KH - 1))
            sq = hepi.tile([KH, 512], f32, tag="sqr")
            # sqrt(x) = exp(0.5*ln(x)) -- keeps ACT on the ln/exp table set
            nc.scalar.activation(sq[:, 0:wdt], nrmp[:, 0:wdt], AF.Ln)
            nc.scalar.activation(sq[:, 0:wdt], sq[:, 0:wdt], AF.Exp,
                                 scale=0.5)
            nc.vector.tensor_scalar(out=sq[:, 0:wdt], in0=sq[:, 0:wdt],
                                    scalar1=cmu_sb[:, 0:1], scalar2=1e-8,
                                    op0=ALU.mult, op1=ALU.max)
            nc.vector.reciprocal(sq[:, 0:wdt], sq[:, 0:wdt])
            res = hepi.tile([KH, 512], f32, tag="res")
            nc.vector.tensor_tensor(res[:, 0:wdt], nump[:, 0:wdt],
                                    sq[:, 0:wdt], op=ALU.mult)
            nc.sync.dma_start(outT[:, st:st + wdt], res[:, 0:wdt])

        _emitted = [0]

        def _post(blk):
            done = (blk + 1) * P
            while _emitted[0] < len(NTL):
                st, wdt = NTL[_emitted[0]]
                if st + wdt > done:
                    break
                head_slice(st, wdt)
                _emitted[0] += 1

        edge_phase(2, post_block=_post)
        while _emitted[0] < len(NTL):
            st, wdt = NTL[_emitted[0]]
            head_slice(st, wdt)
            _emitted[0] += 1
        hstack.close()

    nc.compile()
    return nc


# ======================= host-side preparation ==============================

def _wrap16(flat):
    """idx flat [n] -> wrapped int16 [128, n//16]; pos i -> (i%16, i//16),
    replicated across the 8 Q7-core stripes."""
    n = len(flat)
    out = np.zeros((P, n // 16), np.int16)
    cols = np.arange(n) // 16
    rows = np.arange(n) % 16
    for r in range(8):
        out[r * 16 + rows, cols] = flat
    return out


def _balance_bins(deg, nbins, cap):
    """Greedy multiway partition: assign nodes to bins balancing total degree,
    each bin holding at most `cap` nodes.  Returns bin id per node."""
    import heapq
    n = len(deg)
    order = np.argsort(-deg, kind="stable")
    binid = np.empty(n, np.int32)
    counts = np.zeros(nbins, np.int32)
    heap = [(0, b) for b in range(nbins)]
    heapq.heapify(heap)
    for nd in order:
        while True:
            load, b = heapq.heappop(heap)
            if counts[b] < cap:
                break
        binid[nd] = b
        counts[b] += 1
        if counts[b] < cap:
            heapq.heappush(heap, (load + int(deg[nd]), b))
    return binid


def prep_host(x, edge_index, W1, a_src1, a_dst1, b1, W2, a_src2, a_dst2, b2,
              g, mu, world=8):
    x = np.asarray(x, np.float32)
    N = x.shape[0]
    NBLK = int(np.ceil(N / world / P))
    CAP = NBLK * P
    NT = (N + P - 1) // P
    nbins = world * NBLK

    src = np.asarray(edge_index[0]).astype(np.int64)
    dst = np.asarray(edge_index[1]).astype(np.int64)

    # --- balanced global node -> (core, block, slot) assignment
    deg = np.bincount(dst, minlength=N)
    binid = _balance_bins(deg, nbins, P)
    # slot order within a bin: ascending node id
    order = np.lexsort((np.arange(N), binid))
    gpos = np.empty(N, np.int64)          # node -> global table position
    slot_counts = np.bincount(binid, minlength=nbins)
    bin_start = np.concatenate([[0], np.cumsum(
        np.full(nbins, P, np.int64))])[:-1]
    nxt = bin_start.copy()
    for nd in order:
        b = binid[nd]
        gpos[nd] = nxt[b]
        nxt[b] += 1
    node_core = binid // NBLK
    node_blk = binid % NBLK

    # per-core list of node ids in shard slot order (-1 = empty slot)
    idxmaps = []
    for c in range(world):
        m = np.full(CAP, -1, np.int64)
        mask = node_core == c
        local = gpos[mask] - c * CAP
        m[local] = np.nonzero(mask)[0]
        idxmaps.append(m)

    # --- edges grouped by (core, block) of dst
    ecore = node_core[dst]
    eblk = node_blk[dst]
    gkey = ecore * NBLK + eblk
    gorder = np.argsort(gkey, kind="stable")
    srcg, dstg, gkeyg = src[gorder], dst[gorder], gkey[gorder]
    starts = np.concatenate(
        [[0], np.cumsum(np.bincount(gkeyg, minlength=nbins))])

    # --- per-core permutation: own shard (slot order) first, then the rest
    perms = []
    invs = []
    for c in range(world):
        own = idxmaps[c]
        own_nodes = own[own >= 0]
        other = np.ones(N, bool)
        other[own_nodes] = False
        oth_nodes = np.nonzero(other)[0]
        rest = oth_nodes[np.argsort(gpos[oth_nodes], kind="stable")]
        inv = np.empty(N, np.int64)      # node -> layer-1 table row
        ownslots = gpos[own_nodes] - c * CAP
        inv[own_nodes] = ownslots
        inv[rest] = CAP + np.arange(len(rest))
        perms.append((own_nodes, ownslots, rest))
        invs.append(inv)

    # table rows needed: own-shard slots (CAP) + the non-own nodes
    NT = max(NT, int(np.ceil(
        max(CAP + len(rest_) for (_, _, rest_) in perms) / P)))

    ed = {}
    CPL1 = CPH1 = CPL2 = CPH2 = 1
    for c in range(world):
        inv = invs[c]
        for b in range(NBLK):
            gid = c * NBLK + b
            es = srcg[starts[gid]:starts[gid + 1]]
            eds = dstg[starts[gid]:starts[gid + 1]]
            dloc = (gpos[eds] - c * CAP - b * P).astype(np.int64)
            l1 = inv[es]                   # layer-1 table row (permuted id)
            lo1 = l1 < HALF
            l2 = gpos[es]                  # layer-2 table row (= global pos)
            lo2 = l2 < HALF
            ed[(c, b)] = (l1, lo1, l2, lo2, dloc)
            CPL1 = max(CPL1, int(np.ceil(lo1.sum() / P)))
            CPH1 = max(CPH1, int(np.ceil((~lo1).sum() / P)))
            CPL2 = max(CPL2, int(np.ceil(lo2.sum() / P)))
            CPH2 = max(CPH2, int(np.ceil((~lo2).sum() / P)))

    cfg = CFG(N=N, W=world, NBLK=NBLK, CPL1=CPL1, CPH1=CPH1,
              CPL2=CPL2, CPH2=CPH2, NT=NT, idxmaps=idxmaps)

    def build_layer(c, lnum):
        CPL = CPL1 if lnum == 1 else CPL2
        CPB = cfg.CPB1 if lnum == 1 else cfg.CPB2
        isd = np.zeros((P, NBLK * CPB * 9), np.int16)
        dstf = np.full((P, NBLK * CPB), -1.0, np.float32)  # cast at return
        drow = np.full((1, NBLK * CPB * P), -1.0, np.float32)
        for b in range(NBLK):
            l1, lo1, l2, lo2, dloc = ed[(c, b)]
            ids, lo = (l1, lo1) if lnum == 1 else (l2, lo2)
            fl = np.zeros(CPB * P, np.int64)     # slot -> table idx (pad 0)
            fd = np.full(CPB * P, -1, np.int64)  # slot -> dst_local (pad -1)
            ilo = np.where(lo)[0]
            ihi = np.where(~lo)[0]
            fl[:len(ilo)] = ids[ilo]
            fd[:len(ilo)] = dloc[ilo]
            fl[CPL * P:CPL * P + len(ihi)] = ids[ihi] - HALF
            fd[CPL * P:CPL * P + len(ihi)] = dloc[ihi]
            cb9 = b * CPB * 9
            isd[:, cb9:cb9 + CPB * 8] = _wrap16(fl)
            # dstf[p, j] = fd[j*128 + p], packed after the idx cols
            import ml_dtypes
            dloc_t = fd.reshape(CPB, P).T.astype(np.float32)
            isd[:, cb9 + CPB * 8:cb9 + CPB * 9] = \
                dloc_t.astype(ml_dtypes.bfloat16).view(np.int16)
            drow[0, b * CPB * P:(b + 1) * CPB * P] = fd.astype(np.float32)
        import ml_dtypes
        return isd, drow.astype(ml_dtypes.bfloat16)

    # weights
    W1 = np.asarray(W1, np.float32)
    W2 = np.asarray(W2, np.float32)
    W1r = W1.reshape(H1, MD, IN)
    Ps1 = np.einsum("hdi,hd->ih", W1r, np.asarray(a_src1, np.float32))
    Pd1 = np.einsum("hdi,hd->ih", W1r, np.asarray(a_dst1, np.float32))
    W1aug = np.concatenate([W1.T, Ps1, Pd1], axis=1)
    Ps2 = W2.T @ np.asarray(a_src2, np.float32)[0][:, None]
    Pd2 = W2.T @ np.asarray(a_dst2, np.float32)[0][:, None]
    W2aug = np.concatenate([W2.T, Ps2, Pd2], axis=1)
    AUG1, AUG2 = IN + 4, IN + 2
    w1s = W1aug.reshape(2, P, AUG1).transpose(1, 0, 2).astype(np.float16)
    w2s = W2aug.reshape(2, P, AUG2).transpose(1, 0, 2).astype(np.float16)

    gm = np.asarray(g, np.float32)
    gsd = gm.reshape(2, P, KH * P).transpose(1, 0, 2).astype(np.float16)
    mu = np.asarray(mu, np.float32)
    mus = np.zeros((P, KH * KH), np.float16)
    onesd = np.zeros((P, KH * KH), np.float16)
    for k in range(KH):
        mus[:, k * KH + k] = mu[k, :]
        onesd[:, k * KH + k] = 1.0
    cmu = np.linalg.norm(mu, axis=1)[:, None].astype(np.float32)
    b1b = np.broadcast_to(np.asarray(b1, np.float32), (P, HID)).copy()
    b2b = np.broadcast_to(np.asarray(b2, np.float32), (P, OUT)).copy()
    import ml_dtypes
    iota = np.broadcast_to(np.arange(P, dtype=np.float32),
                           (P, P)).astype(ml_dtypes.bfloat16)
    iotac = np.arange(P, dtype=np.float32)[:, None]
    ones1 = np.ones((1, P), ml_dtypes.bfloat16)
    ident = np.eye(P, dtype=np.float32)

    Npad = NT * P
    shared = dict(w1s=w1s, w2s=w2s, gs=gsd, mus=mus, onesd=onesd, cmu=cmu,
                  b1b=b1b, b2b=b2b, iota=iota, iotac=iotac, ones1=ones1,
                  ident=ident)
    in_maps = []
    for c in range(world):
        own_nodes, ownslots, rest = perms[c]
        xp = np.zeros((Npad, IN), np.float32)
        xp[ownslots] = x[own_nodes]
        xp[CAP:CAP + len(rest)] = x[rest]
        xTi = xp.reshape(NT, P, 2, P).transpose(3, 0, 2, 1).astype(np.float16)
        i1, r1 = build_layer(c, 1)
        i2, r2 = build_layer(c, 2)
        m = dict(shared)
        m.update(xTi=xTi, isd1=i1, dstrow1=r1, isd2=i2, dstrow2=r2)
        in_maps.append(m)
    return cfg, in_maps


def assemble(cfg, outs):
    N = cfg.N
    full = np.zeros((N, KH), np.float32)
    for c in range(cfg.W):
        o = np.asarray(outs[c]["outT"])      # [KH, SHARD_CAP]
        m = cfg.idxmaps[c]
        valid = m >= 0
        full[m[valid], :] = o[:, valid].T
    return full


_CACHE = {}


def kernel(**inputs):
    world = 8
    cfg, in_maps = prep_host(world=world, **inputs)
    key = (cfg.N, cfg.W, cfg.CPB1, cfg.CPB2)
    if key not in _CACHE:
        _CACHE[key] = build_program(cfg)
    nc = _CACHE[key]

    from concourse.bass_utils import run_bass_kernel_spmd
    res = run_bass_kernel_spmd(nc, in_maps, core_ids=list(range(world)))
    return assemble(cfg, res.results)


# revision 40
# speedup vs baseline: 1.0005x; 1.0005x over previous
"""Trainium2 Bass kernel for nn_NodeInference (2-layer GAT + cosine head).

v2 design (SPMD over 8 cores, dst-node sharding):
  Host globally re-assigns nodes to (core, block) bins, balancing per-block
  in-degree so chunk capacities carry minimal padding. Self-loops are NOT in
  the edge lists; each block handles them as one extra "virtual chunk" fed
  directly from the block's own 128-row window (no gather).

  Packed row tables in HBM, 384 fp16 cols (768B, dma_gather-compatible):
     L1 row = [h fp16 x256 | a_src f32 x2 | a_dst f32 x2 | pad]
     L2 row = [h fp16 x256 | a_src f32 | a_dst f32 | pad]
  P1  dense-1 (replicated, permuted order): h1aug = x @ W1aug -> h1 table
  P2  edge phase (per dst block of 128 nodes):
      - direct window load of the block's own rows (a_dst source + self loop)
      - bulk src-row gather via dma_gather (int16 idx; lo/hi table halves)
      - PE broadcast of per-chunk dst-locals (ones[1,128]^T @ dstrow) ->
        S_T[d,e] = is_equal(bcast, iota_col); a_d per edge = S_T^T @ adwin
      - w_e = exp(min(leakyrelu(a_s + a_d, 0.2), 30))
      - Sw_h[e,d] = (iota == dstloc) * w_h  (fused DVE op); PSUM accum:
        bp[:,hP:(h+1)P] += Sw_h^T @ h_rows ; bp[:,2P+h] += Sw_h^T @ ones
      - out1 = leakyrelu(bp_h/SumW_h + b1, 0.01); PE-transpose -> out1T
      - h2aug = out1 @ W2aug -> packed rows -> cc_in (local shard rows)
  P3  AllGather cc_in -> cc_out (row index == global node position)
  P4  edge phase 2 (1 head) over cc_out/cc_in
  P5  head: cos sim vs mu -> outT [8, SHARD_CAP]
Host scatters per-core outT into the full output via the assignment map.
"""

import sys
from dataclasses import dataclass, field
from contextlib import ExitStack

if "/opt/trn_rl_repo" not in sys.path:
    sys.path.insert(0, "/opt/trn_rl_repo")

import numpy as np

import concourse.bacc as bacc
import concourse.bass as bass
import concourse.mybir as mybir
import concourse.tile as tile
from concourse.bass import AP

P = 128
IN = 256          # input feature dim
H1 = 2            # layer-1 heads
HID = 256         # layer-1 output dim (2*128, concat)
OUT = 256         # layer-2 output dim
KH, MD = 8, 128   # cosine head shape
ROWW = 384        # fp16 cols per packed table row (768B)
HALF = 32768      # int16 table-half split
AF = mybir.ActivationFunctionType
ALU = mybir.AluOpType
DT = mybir.dt
USE_COLLECTIVE = True


@dataclass
class CFG:
    N: int
    W: int              # world size
    NBLK: int           # dst blocks (128 dsts) per core
    CPL1: int           # lo-half chunks per block, layer 1
    CPH1: int
    CPL2: int
    CPH2: int
    NT: int             # dense-1 node tiles
    idxmaps: object = field(default=None, repr=False)

    @property
    def Npad(self):
        return self.NT * P

    @property
    def SHARD_CAP(self):
        return self.NBLK * P

    @property
    def CPB1(self):
        return self.CPL1 + self.CPH1

    @property
    def CPB2(self):
        return self.CPL2 + self.CPH2


def build_program(cfg: CFG):
    nc = bacc.Bacc("TRN2", target_bir_lowering=False, debug=False)
    W, NBLK = cfg.W, cfg.NBLK
    AUG1, AUG2 = IN + 4, IN + 2
    f16, bf16, f32 = DT.float16, DT.bfloat16, DT.float32
    i16 = DT.int16

    with tile.TileContext(nc) as tc, ExitStack() as stack:
        dram = stack.enter_context(
            tc.tile_pool(name="dram", bufs=1, space="DRAM"))

        def din(name, shape, dtype):
            return dram.tile(shape, dtype, kind="ExternalInput", name=name,
                             uniquify=False)

        xTi = din("xTi", [P, cfg.NT, 2, P], f16)
        w1s = din("w1s", [P, 2, AUG1], f16)
        w2s = din("w2s", [P, 2, AUG2], f16)
        gsd = din("gs", [P, 2, KH * P], f16)
        mus = din("mus", [P, KH * KH], f16)       # block-diag mu^T
        ond = din("onesd", [P, KH * KH], f16)     # block-diag ones
        cmu = din("cmu", [KH, 1], f32)
        b1d = din("b1b", [P, HID], f32)
        b2d = din("b2b", [P, OUT], f32)
        iot = din("iota", [P, P], bf16)
        ioc = din("iotac", [P, 1], f32)           # iota column (partition idx)
        one1 = din("ones1", [1, P], bf16)         # bcast matmul lhsT
        idn = din("ident", [P, P], f32)
        is1 = din("isd1", [P, NBLK * cfg.CPB1 * 9], i16)
        dr1 = din("dstrow1", [1, NBLK * cfg.CPB1 * P], bf16)
        is2 = din("isd2", [P, NBLK * cfg.CPB2 * 9], i16)
        dr2 = din("dstrow2", [1, NBLK * cfg.CPB2 * P], bf16)
        outT = dram.tile([KH, cfg.SHARD_CAP], f32, kind="ExternalOutput",
                         name="outT", uniquify=False)

        h1lo = dram.tile([min(HALF, cfg.Npad), ROWW], f16,
                         name="h1_lo")
        h1hi = dram.tile([max(cfg.Npad - HALF, P), ROWW], f16,
                         name="h1_hi")
        cc_in = dram.tile([cfg.SHARD_CAP, ROWW], f16, name="cc_in")
        cc_out = dram.tile([W * cfg.SHARD_CAP, ROWW], f16, name="cc_out",
                           addr_space="Shared" if W > 1 else "Local")

        consts = stack.enter_context(tc.tile_pool(name="consts", bufs=1))
        w1_sb = consts.tile([P, 2, AUG1], f16)
        w2_sb = consts.tile([P, 2, AUG2], f16)
        g_sb = consts.tile([P, 2, KH * P], f16)
        mu_sb = consts.tile([P, KH * KH], f16)
        on_sb = consts.tile([P, KH * KH], f16)
        cmu_sb = consts.tile([KH, 1], f32)
        b1_sb = consts.tile([P, HID], f32)
        b2_sb = consts.tile([P, OUT], f32)
        iota_sb = consts.tile([P, P], bf16)
        iotac_sb = consts.tile([P, 1], f32)
        ones1_sb = consts.tile([1, P], bf16)
        ident_sb = consts.tile([P, P], f32)
        out1T_sb = consts.tile([P, 2, cfg.SHARD_CAP], f16)
        h2fT_sb = consts.tile([P, 2, cfg.SHARD_CAP], f16)

        for dst, src in [(w1_sb, w1s), (w2_sb, w2s), (g_sb, gsd),
                         (mu_sb, mus), (on_sb, ond), (cmu_sb, cmu),
                         (b1_sb, b1d), (b2_sb, b2d), (iota_sb, iot),
                         (iotac_sb, ioc), (ones1_sb, one1),
                         (ident_sb, idn)]:
            nc.sync.dma_start(dst[:], src[:])

        if cfg.Npad <= HALF:
            # h1hi is a dummy (P1 never writes it); zero it so padding
            # gathers read finite values.
            zhi = consts.tile([P, ROWW], f16)
            nc.vector.memset(zhi[:], 0.0)
            nc.sync.dma_start(h1hi[0:P, :], zhi[:])

        # ================= P1: dense layer 1 (replicated, permuted) =========
        # 4 tiles per DMA batch: HWDGE dispatch (~0.6us/dma) dominates at
        # one-tile granularity.
        TB = 4
        with tc.tile_pool(name="p1x", bufs=3) as p1x, \
             tc.tile_pool(name="p1ps", bufs=4, space="PSUM") as p1ps, \
             tc.tile_pool(name="p1row", bufs=3) as p1row:
            for t0 in range(0, cfg.NT, TB):
                tb = min(TB, cfg.NT - t0)
                xt = p1x.tile([P, TB, 2, P], f16, tag="xt")
                nc.sync.dma_start(xt[:, 0:tb, :, :], xTi[:, t0:t0 + tb, :, :])
                # row layout: [h1 0:128 | 1.0 | h2 129:257 | 1.0 |
                #              a_s f32 @129:131 | a_d f32 @131:133]
                row = p1row.tile([P, TB, ROWW], f16, tag="row")
                rf32 = row[:].bitcast(f32)
                for i in range(tb):
                    ps = p1ps.tile([P, AUG1], f32, tag="ps")
                    for k in range(2):
                        nc.tensor.matmul(ps[:], lhsT=xt[:, i, k, :],
                                         rhs=w1_sb[:, k, :],
                                         start=(k == 0), stop=(k == 1))
                    nc.scalar.activation(row[:, i, 0:P], ps[:, 0:P], AF.Copy)
                    nc.scalar.activation(row[:, i, P + 1:IN + 1],
                                         ps[:, P:IN], AF.Copy)
                    nc.vector.tensor_copy(rf32[:, i, 129:133],
                                          ps[:, IN:IN + 4])
                nc.vector.memset(row[:, 0:tb, P:P + 1], 1.0)
                nc.vector.memset(row[:, 0:tb, IN + 1:IN + 2], 1.0)
                # tiles t<HALF//P land in h1lo, the rest in h1hi (TB
                # divides HALF//P, so a batch never straddles)
                if t0 * P < HALF:
                    hap, off = h1lo[:], t0 * P * ROWW
                else:
                    hap, off = h1hi[:], (t0 * P - HALF) * ROWW
                dst = AP(hap.tensor, off,
                         [[ROWW, P], [P * ROWW, tb], [1, IN + 10]])
                nc.sync.dma_start(dst, row[:, 0:tb, 0:IN + 10])

        # ================= P2/P4: edge phases ================================
        def edge_phase(layer, post_block=None):
            if layer == 1:
                tab_lo, tab_hi = h1lo[:, :], h1hi[:, :]

                def dst_win(blk):
                    return h1lo[blk * P:(blk + 1) * P, :]
                isrc_d, drow_d = is1, dr1
                CPL, CPH, CPB = cfg.CPL1, cfg.CPH1, cfg.CPB1
                nhead = 2
                b_sb, out_t, lrelu_out = b1_sb, out1T_sb, True
                as_off, ad_off = 129, 131   # f32 col offsets
            else:
                ccrows = W * cfg.SHARD_CAP
                tab_lo = cc_out[0:min(HALF, ccrows), :]
                tab_hi = (cc_out[HALF:ccrows, :] if ccrows > HALF
                          else tab_lo)

                def dst_win(blk):
                    return cc_in[blk * P:(blk + 1) * P, :]
                isrc_d, drow_d = is2, dr2
                CPL, CPH, CPB = cfg.CPL2, cfg.CPH2, cfg.CPB2
                nhead = 1
                b_sb, out_t, lrelu_out = b2_sb, h2fT_sb, False
                as_off, ad_off = 129, 130

            estack = ExitStack()
            pi = estack.enter_context(
                tc.tile_pool(name=f"idx{layer}", bufs=5))
            pg = estack.enter_context(
                tc.tile_pool(name=f"gath{layer}", bufs=4))
            pw = estack.enter_context(
                tc.tile_pool(name=f"win{layer}", bufs=5))
            pb = estack.enter_context(
                tc.tile_pool(name=f"bcps{layer}", bufs=1, space="PSUM"))
            pst = estack.enter_context(tc.tile_pool(name=f"st{layer}", bufs=3))
            pad_ = estack.enter_context(
                tc.tile_pool(name=f"adps{layer}", bufs=1, space="PSUM"))
            pe_ = estack.enter_context(tc.tile_pool(name=f"ew{layer}", bufs=2))
            pm = estack.enter_context(tc.tile_pool(name=f"sw{layer}", bufs=2))
            pp = estack.enter_context(
                tc.tile_pool(name=f"bps{layer}", bufs=2, space="PSUM"))
            pt = estack.enter_context(
                tc.tile_pool(name=f"tps{layer}", bufs=1, space="PSUM"))
            po = estack.enter_context(tc.tile_pool(name=f"epi{layer}", bufs=2))
            ph = estack.enter_context(
                tc.tile_pool(name=f"h2ps{layer}", bufs=1, space="PSUM"))

            NSW = CPB + 1   # chunks + self-loop virtual chunk

            # Software-pipelined gathers: lo-half gathers for block b run
            # LAG blocks ahead of processing, so they overlap the producer
            # phase's tail (h1hi still being written) and keep Pool fed.
            LAG = 3
            MXC = 8
            state = {}

            def emit_front(blk):
                cb9 = blk * CPB * 9
                isd = pi.tile([P, CPB * 9], i16, tag="isd")
                drow = pi.tile([1, CPB * P], bf16, tag="drow")
                nc.sync.dma_start(isd[:], isrc_d[:, cb9:cb9 + CPB * 9])
                nc.sync.dma_start(drow[:], drow_d[:, blk * CPB * P:
                                                  (blk + 1) * CPB * P])
                isrc = isd[:, 0:CPB * 8]
                win = pw.tile([P, ROWW], f16, tag="win")
                nc.sync.dma_start(win[:], dst_win(blk))
                gt = pg.tile([P, CPB, ROWW], f16, tag="gt")
                for c0 in range(0, CPL, MXC):
                    c1 = min(c0 + MXC, CPL)
                    nc.gpsimd.dma_gather(
                        gt[:, c0:c1, :], tab_lo,
                        isrc[:, c0 * 8:c1 * 8],
                        (c1 - c0) * P, (c1 - c0) * P, ROWW)
                state[blk] = (isd, drow, win, gt)

            def emit_hi(blk):
                isd, drow, win, gt = state[blk]
                isrc = isd[:, 0:CPB * 8]
                for c0 in range(CPL, CPB, MXC):
                    c1 = min(c0 + MXC, CPB)
                    nc.gpsimd.dma_gather(
                        gt[:, c0:c1, :], tab_hi,
                        isrc[:, c0 * 8:c1 * 8],
                        (c1 - c0) * P, (c1 - c0) * P, ROWW)

            for pipei in range(NBLK + LAG):
                if pipei < NBLK:
                    emit_front(pipei)
                if pipei < LAG:
                    continue
                blk = pipei - LAG
                emit_hi(blk)
                isd, drow, win, gt = state.pop(blk)
                dstf = isd[:, CPB * 8:CPB * 9].bitcast(bf16)
                winf = win[:].bitcast(f32)
                gtf = gt[:].bitcast(f32)

                # --- dst-local broadcast (PE) -> dstb (ACT copy) -> S_T via
                # one batched DVE is_equal over all chunks
                dstb = pst.tile([P, CPB * P], bf16, tag="dstb")
                st = pst.tile([P, CPB * P], bf16, tag="st")
                GW = 4  # chunks per broadcast matmul (512 psum cols)
                for g0 in range(0, CPB, GW):
                    g1 = min(g0 + GW, CPB)
                    bc = pb.tile([P, GW * P], f32, tag="bc")
                    nc.tensor.matmul(bc[:, 0:(g1 - g0) * P],
                                     lhsT=ones1_sb[:],
                                     rhs=drow[:, g0 * P:g1 * P],
                                     start=True, stop=True)
                    nc.scalar.activation(dstb[:, g0 * P:g1 * P],
                                         bc[:, 0:(g1 - g0) * P], AF.Copy)
                nc.vector.tensor_scalar(
                    out=st[:], in0=dstb[:], scalar1=iotac_sb[:, 0:1],
                    scalar2=None, op0=ALU.is_equal)

                # --- a_d per edge: ad_ps[e, (j,h)] = S_T_j^T @ adwin
                adw16 = pe_.tile([P, nhead], f16, tag="adw16")
                nc.vector.tensor_copy(adw16[:], winf[:, ad_off:ad_off + nhead])
                # one accumulation group for the whole tile: start=True zeroes
                # the full 2KB psum region, so only the first matmul may start
                # and only the last may stop; disjoint writes in between are
                # first-touch overwrites.
                ad_ps = pad_.tile([P, CPB * nhead], f32, tag="adps")
                for j in range(CPB):
                    nc.tensor.matmul(ad_ps[:, j * nhead:(j + 1) * nhead],
                                     lhsT=st[:, j * P:(j + 1) * P],
                                     rhs=adw16[:], start=(j == 0),
                                     stop=(j == CPB - 1))

                # --- edge weights w = exp(lrelu(a_s + a_d, .2))
                ew = pe_.tile([P, CPB * nhead], f32, tag="ew")
                wv = pe_.tile([P, CPB * nhead], bf16, tag="wv")
                as_v = gtf[:, :, as_off:as_off + nhead]
                nc.vector.tensor_tensor(
                    ew[:], as_v, ad_ps[:], op=ALU.add)
                ewl = pe_.tile([P, CPB * nhead], f32, tag="ewl")
                nc.vector.tensor_scalar(out=ewl[:], in0=ew[:], scalar1=0.2,
                                        scalar2=None, op0=ALU.mult)
                nc.vector.tensor_tensor(ewl[:], ewl[:], ew[:], op=ALU.max)
                nc.scalar.activation(wv[:], ewl[:], AF.Exp)

                # --- self-loop weights from the window rows
                ws = pe_.tile([P, nhead], f32, tag="ws")
                nc.vector.tensor_tensor(
                    ws[:], winf[:, as_off:as_off + nhead],
                    winf[:, ad_off:ad_off + nhead], op=ALU.add)
                wt = pe_.tile([P, nhead], f32, tag="wt")
                nc.vector.tensor_scalar(out=wt[:], in0=ws[:], scalar1=0.2,
                                        scalar2=None, op0=ALU.mult)
                nc.vector.tensor_tensor(wt[:], wt[:], ws[:], op=ALU.max)
                nc.scalar.activation(wt[:], wt[:], AF.Exp)

                # --- batched one-hot scatter matrices:
                #     s_all[e,(j,d)] = (dstf[e,j] == iota[d])
                #     sw_h[e,(j,d)]  = s_all * w_h[e,j]   (stride-0 bcasts)
                sall = pm.tile([P, CPB * P], bf16, tag="sall")
                dfa = dstf[:, 0:1]
                dview = AP(dfa.tensor, dfa.offset,
                           [dfa.ap[0], [1, CPB], [0, P]])
                ioa = iota_sb[:]
                iview = AP(ioa.tensor, ioa.offset,
                           [ioa.ap[0], [0, CPB], [1, P]])
                nc.vector.tensor_tensor(sall[:], dview, iview,
                                        op=ALU.is_equal)
                swh = []
                for h in range(nhead):
                    wvh = wv[:, h:h + 1]
                    wview = AP(wvh.tensor, wvh.offset,
                               [wvh.ap[0], [nhead, CPB], [0, P]])
                    swt = pm.tile([P, CPB * P], bf16, tag=f"swh{h}")
                    nc.vector.tensor_tensor(swt[:], sall[:], wview,
                                            op=ALU.mult)
                    swh.append(swt)

                # --- weighted one-hot scatter accumulation
                # single psum accumulation group across all j/h matmuls (see
                # zero-region note above): start only on the first matmul,
                # stop only on the very last.  The table rows carry a literal
                # 1.0 after each head's features, so one matmul accumulates
                # both the weighted feature sum and the softmax denominator:
                # bp layout [f1 0:128 | sumw1 | f2 129:257 | sumw2] (2 heads)
                # or [f 0:256 | sumw] (1 head).
                HB = P + 1 if nhead == 2 else IN + 1
                bp = pp.tile([P, nhead * HB], f32, tag="bp")
                for j in range(NSW):
                    last = (j == NSW - 1)
                    selfc = (j == CPB)
                    for h in range(nhead):
                        if selfc:
                            sw = pm.tile([P, P], bf16, tag="sw")
                            nc.vector.tensor_scalar(
                                out=sw[:], in0=iota_sb[:],
                                scalar1=iotac_sb[:, 0:1],
                                scalar2=wt[:, h:h + 1],
                                op0=ALU.is_equal, op1=ALU.mult)
                            lhsT = sw[:]
                        else:
                            lhsT = swh[h][:, j * P:(j + 1) * P]
                        c0, c1 = h * HB, (h + 1) * HB
                        rhs = win[:, c0:c1] if selfc else gt[:, j, c0:c1]
                        nc.tensor.matmul(bp[:, c0:c1], lhsT=lhsT,
                                         rhs=rhs,
                                         start=(j == 0 and h == 0),
                                         stop=(last and h == nhead - 1))

                # ---- block epilogue
                rec = po.tile([P, nhead], f32, tag="rec")
                for h in range(nhead):
                    nc.vector.reciprocal(rec[:, h:h + 1],
                                         bp[:, (h + 1) * HB - 1:
                                            (h + 1) * HB])
                ti = po.tile([P, IN], f32, tag="ti")
                if nhead == 2:
                    nc.scalar.activation(ti[:, 0:P], bp[:, 0:P], AF.Copy,
                                         scale=rec[:, 0:1])
                    nc.scalar.activation(ti[:, P:IN], bp[:, HB:HB + P],
                                         AF.Copy, scale=rec[:, 1:2])
                else:
                    nc.scalar.activation(ti[:], bp[:, 0:IN], AF.Copy,
                                         scale=rec[:, 0:1])
                nc.vector.tensor_tensor(ti[:], ti[:], b_sb[:], op=ALU.add)
                if lrelu_out:
                    # lrelu(x) = 0.01x + relu(0.99x)
                    tr = po.tile([P, IN], f32, tag="tr")
                    nc.scalar.activation(tr[:], ti[:], AF.Relu, scale=0.99)
                    nc.vector.scalar_tensor_tensor(
                        out=ti[:], in0=ti[:], scalar=0.01, in1=tr[:],
                        op0=ALU.mult, op1=ALU.add)
                for k in range(2):
                    tp = pt.tile([P, P], f32, tag="tp")
                    nc.tensor.transpose(tp[:], ti[:, k * P:(k + 1) * P],
                                        ident_sb[:])
                    nc.scalar.activation(out_t[:, k, blk * P:(blk + 1) * P],
                                         tp[:], AF.Copy)

                if layer == 1:
                    hp = ph.tile([P, AUG2], f32, tag="hp")
                    for k in range(2):
                        nc.tensor.matmul(
                            hp[:],
                            lhsT=out1T_sb[:, k, blk * P:(blk + 1) * P],
                            rhs=w2_sb[:, k, :], start=(k == 0), stop=(k == 1))
                    # L2 row: [h 0:256 | 1.0 | a_s f32 @129 | a_d f32 @130]
                    row2 = po.tile([P, ROWW], f16, tag="row2")
                    nc.scalar.activation(row2[:, 0:OUT], hp[:, 0:OUT], AF.Copy)
                    nc.vector.memset(row2[:, OUT:OUT + 2], 1.0)
                    r2f = row2[:].bitcast(f32)
                    nc.vector.tensor_copy(r2f[:, 129:131],
                                          hp[:, OUT:OUT + 2])
                    nc.sync.dma_start(
                        cc_in[blk * P:(blk + 1) * P, 0:OUT + 6],
                        row2[:, 0:OUT + 6])

                if post_block is not None:
                    post_block(blk)

            estack.close()

        edge_phase(1)

        nc.gpsimd.collective_compute(
            "AllGather", ALU.bypass,
            replica_groups=[list(range(W))],
            ins=[cc_in[:]], outs=[cc_out[:]])

        # ====== P5 head, interleaved into edge phase 2 ======================
        hstack = ExitStack()
        hps = hstack.enter_context(
            tc.tile_pool(name="hps", bufs=1, space="PSUM"))
        hsb = hstack.enter_context(tc.tile_pool(name="hsb", bufs=2))
        sps = hstack.enter_context(
            tc.tile_pool(name="sps", bufs=1, space="PSUM"))
        hepi = hstack.enter_context(tc.tile_pool(name="hepi", bufs=2))

        NTL = []
        _st = 0
        while _st < cfg.SHARD_CAP:
            _w = min(512, cfg.SHARD_CAP - _st)
            NTL.append((_st, _w))
            _st += _w

        def head_slice(st, wdt):
            nump = sps.tile([KH, 512], f32, tag="nump")
            nrmp = sps.tile([KH, 512], f32, tag="nrmp")
            for k in range(KH):
                hp = hps.tile([P, 512], f32, tag="hp")
                for f in range(2):
                    nc.tensor.matmul(hp[:, 0:wdt],
                                     lhsT=g_sb[:, f, k * P:(k + 1) * P],
                                     rhs=h2fT_sb[:, f, st:st + wdt],
                                     start=(f == 0), stop=(f == 1))
                h16 = hsb.tile([P, 512], f16, tag="h16")
                sq16 = hsb.tile([P, 512], f16, tag="sq16")
                nc.vector.tensor_copy(h16[:, 0:wdt], hp[:, 0:wdt])
                nc.scalar.activation(sq16[:, 0:wdt], hp[:, 0:wdt], AF.Square)
                nc.tensor.matmul(nump[:, 0:wdt],
                                 lhsT=mu_sb[:, k * KH:(k + 1) * KH],
                                 rhs=h16[:, 0:wdt], start=(k == 0),
                                 stop=(k == KH - 1))
                nc.tensor.matmul(nrmp[:, 0:wdt],
                                 lhsT=on_sb[:, k * KH:(k + 1) * KH],
                                 rhs=sq16[:, 0:wdt], start=(k == 0),
                                 stop=(k == KH - 1))
            sq = hepi.tile([KH, 512], f32, tag="sqr")
            # sqrt(x) = exp(0.5*ln(x)) -- keeps ACT on the ln/exp table set
            nc.scalar.activation(sq[:, 0:wdt], nrmp[:, 0:wdt], AF.Ln)
            nc.scalar.activation(sq[:, 0:wdt], sq[:, 0:wdt], AF.Exp,
                                 scale=0.5)
            nc.vector.tensor_scalar(out=sq[:, 0:wdt], in0=sq[:, 0:wdt],
                                    scalar1=cmu_sb[:, 0:1], scalar2=1e-8,
                                    op0=ALU.mult, op1=ALU.max)
            nc.vector.reciprocal(sq[:, 0:wdt], sq[:, 0:wdt])
            res = hepi.tile([KH, 512], f32, tag="res")
            nc.vector.tensor_tensor(res[:, 0:wdt], nump[:, 0:wdt],
                                    sq[:, 0:wdt], op=ALU.mult)
            nc.sync.dma_start(outT[:, st:st + wdt], res[:, 0:wdt])

        _emitted = [0]

        def _post(blk):
            done = (blk + 1) * P
            while _emitted[0] < len(NTL):
                st, wdt = NTL[_emitted[0]]
                if st + wdt > done:
                    break
                head_slice(st, wdt)
                _emitted[0] += 1

        edge_phase(2, post_block=_post)
        while _emitted[0] < len(NTL):
            st, wdt = NTL[_emitted[0]]
            head_slice(st, wdt)
            _emitted[0] += 1
        hstack.close()

    nc.compile()
    return nc


# ======================= host-side preparation ==============================

def _wrap16(flat):
    """idx flat [n] -> wrapped int16 [128, n//16]; pos i -> (i%16, i//16),
    replicated across the 8 Q7-core stripes."""
    n = len(flat)
    out = np.zeros((P, n // 16), np.int16)
    cols = np.arange(n) // 16
    rows = np.arange(n) % 16
    for r in range(8):
        out[r * 16 + rows, cols] = flat
    return out


def _balance_bins(deg, nbins, cap):
    """Greedy multiway partition: assign nodes to bins balancing total degree,
    each bin holding at most `cap` nodes.  Returns bin id per node."""
    import heapq
    n = len(deg)
    order = np.argsort(-deg, kind="stable")
    binid = np.empty(n, np.int32)
    counts = np.zeros(nbins, np.int32)
    heap = [(0, b) for b in range(nbins)]
    heapq.heapify(heap)
    for nd in order:
        while True:
            load, b = heapq.heappop(heap)
            if counts[b] < cap:
                break
        binid[nd] = b
        counts[b] += 1
        if counts[b] < cap:
            heapq.heappush(heap, (load + int(deg[nd]), b))
    return binid


def prep_host(x, edge_index, W1, a_src1, a_dst1, b1, W2, a_src2, a_dst2, b2,
              g, mu, world=8):
    x = np.asarray(x, np.float32)
    N = x.shape[0]
    NBLK = int(np.ceil(N / world / P))
    CAP = NBLK * P
    NT = (N + P - 1) // P
    nbins = world * NBLK

    src = np.asarray(edge_index[0]).astype(np.int64)
    dst = np.asarray(edge_index[1]).astype(np.int64)

    # --- balanced global node -> (core, block, slot) assignment
    deg = np.bincount(dst, minlength=N)
    binid = _balance_bins(deg, nbins, P)
    # slot order within a bin: ascending node id
    order = np.lexsort((np.arange(N), binid))
    gpos = np.empty(N, np.int64)          # node -> global table position
    slot_counts = np.bincount(binid, minlength=nbins)
    bin_start = np.concatenate([[0], np.cumsum(
        np.full(nbins, P, np.int64))])[:-1]
    nxt = bin_start.copy()
    for nd in order:
        b = binid[nd]
        gpos[nd] = nxt[b]
        nxt[b] += 1
    node_core = binid // NBLK
    node_blk = binid % NBLK

    # per-core list of node ids in shard slot order (-1 = empty slot)
    idxmaps = []
    for c in range(world):
        m = np.full(CAP, -1, np.int64)
        mask = node_core == c
        local = gpos[mask] - c * CAP
        m[local] = np.nonzero(mask)[0]
        idxmaps.append(m)

    # --- edges grouped by (core, block) of dst
    ecore = node_core[dst]
    eblk = node_blk[dst]
    gkey = ecore * NBLK + eblk
    gorder = np.argsort(gkey, kind="stable")
    srcg, dstg, gkeyg = src[gorder], dst[gorder], gkey[gorder]
    starts = np.concatenate(
        [[0], np.cumsum(np.bincount(gkeyg, minlength=nbins))])

    # --- per-core permutation: own shard (slot order) first, then the rest
    perms = []
    invs = []
    for c in range(world):
        own = idxmaps[c]
        own_nodes = own[own >= 0]
        other = np.ones(N, bool)
        other[own_nodes] = False
        oth_nodes = np.nonzero(other)[0]
        rest = oth_nodes[np.argsort(gpos[oth_nodes], kind="stable")]
        inv = np.empty(N, np.int64)      # node -> layer-1 table row
        ownslots = gpos[own_nodes] - c * CAP
        inv[own_nodes] = ownslots
        inv[rest] = CAP + np.arange(len(rest))
        perms.append((own_nodes, ownslots, rest))
        invs.append(inv)

    # table rows needed: own-shard slots (CAP) + the non-own nodes
    NT = max(NT, int(np.ceil(
        max(CAP + len(rest_) for (_, _, rest_) in perms) / P)))

    ed = {}
    CPL1 = CPH1 = CPL2 = CPH2 = 1
    for c in range(world):
        inv = invs[c]
        for b in range(NBLK):
            gid = c * NBLK + b
            es = srcg[starts[gid]:starts[gid + 1]]
            eds = dstg[starts[gid]:starts[gid + 1]]
            dloc = (gpos[eds] - c * CAP - b * P).astype(np.int64)
            l1 = inv[es]                   # layer-1 table row (permuted id)
            lo1 = l1 < HALF
            l2 = gpos[es]                  # layer-2 table row (= global pos)
            lo2 = l2 < HALF
            ed[(c, b)] = (l1, lo1, l2, lo2, dloc)
            CPL1 = max(CPL1, int(np.ceil(lo1.sum() / P)))
            CPH1 = max(CPH1, int(np.ceil((~lo1).sum() / P)))
            CPL2 = max(CPL2, int(np.ceil(lo2.sum() / P)))
            CPH2 = max(CPH2, int(np.ceil((~lo2).sum() / P)))

    cfg = CFG(N=N, W=world, NBLK=NBLK, CPL1=CPL1, CPH1=CPH1,
              CPL2=CPL2, CPH2=CPH2, NT=NT, idxmaps=idxmaps)

    def build_layer(c, lnum):
        CPL = CPL1 if lnum == 1 else CPL2
        CPB = cfg.CPB1 if lnum == 1 else cfg.CPB2
        isd = np.zeros((P, NBLK * CPB * 9), np.int16)
        dstf = np.full((P, NBLK * CPB), -1.0, np.float32)  # cast at return
        drow = np.full((1, NBLK * CPB * P), -1.0, np.float32)
        for b in range(NBLK):
            l1, lo1, l2, lo2, dloc = ed[(c, b)]
            ids, lo = (l1, lo1) if lnum == 1 else (l2, lo2)
            fl = np.zeros(CPB * P, np.int64)     # slot -> table idx (pad 0)
            fd = np.full(CPB * P, -1, np.int64)  # slot -> dst_local (pad -1)
            ilo = np.where(lo)[0]
            ihi = np.where(~lo)[0]
            fl[:len(ilo)] = ids[ilo]
            fd[:len(ilo)] = dloc[ilo]
            fl[CPL * P:CPL * P + len(ihi)] = ids[ihi] - HALF
            fd[CPL * P:CPL * P + len(ihi)] = dloc[ihi]
            cb9 = b * CPB * 9
            isd[:, cb9:cb9 + CPB * 8] = _wrap16(fl)
            # dstf[p, j] = fd[j*128 + p], packed after the idx cols
            import ml_dtypes
            dloc_t = fd.reshape(CPB, P).T.astype(np.float32)
            isd[:, cb9 + CPB * 8:cb9 + CPB * 9] = \
                dloc_t.astype(ml_dtypes.bfloat16).view(np.int16)
            drow[0, b * CPB * P:(b + 1) * CPB * P] = fd.astype(np.float32)
        import ml_dtypes
        return isd, drow.astype(ml_dtypes.bfloat16)

    # weights
    W1 = np.asarray(W1, np.float32)
    W2 = np.asarray(W2, np.float32)
    W1r = W1.reshape(H1, MD, IN)
    Ps1 = np.einsum("hdi,hd->ih", W1r, np.asarray(a_src1, np.float32))
    Pd1 = np.einsum("hdi,hd->ih", W1r, np.asarray(a_dst1, np.float32))
    W1aug = np.concatenate([W1.T, Ps1, Pd1], axis=1)
    Ps2 = W2.T @ np.asarray(a_src2, np.float32)[0][:, None]
    Pd2 = W2.T @ np.asarray(a_dst2, np.float32)[0][:, None]
    W2aug = np.concatenate([W2.T, Ps2, Pd2], axis=1)
    AUG1, AUG2 = IN + 4, IN + 2
    w1s = W1aug.reshape(2, P, AUG1).transpose(1, 0, 2).astype(np.float16)
    w2s = W2aug.reshape(2, P, AUG2).transpose(1, 0, 2).astype(np.float16)

    gm = np.asarray(g, np.float32)
    gsd = gm.reshape(2, P, KH * P).transpose(1, 0, 2).astype(np.float16)
    mu = np.asarray(mu, np.float32)
    mus = np.zeros((P, KH * KH), np.float16)
    onesd = np.zeros((P, KH * KH), np.float16)
    for k in range(KH):
        mus[:, k * KH + k] = mu[k, :]
        onesd[:, k * KH + k] = 1.0
    cmu = np.linalg.norm(mu, axis=1)[:, None].astype(np.float32)
    b1b = np.broadcast_to(np.asarray(b1, np.float32), (P, HID)).copy()
    b2b = np.broadcast_to(np.asarray(b2, np.float32), (P, OUT)).copy()
    import ml_dtypes
    iota = np.broadcast_to(np.arange(P, dtype=np.float32),
                           (P, P)).astype(ml_dtypes.bfloat16)
    iotac = np.arange(P, dtype=np.float32)[:, None]
    ones1 = np.ones((1, P), ml_dtypes.bfloat16)
    ident = np.eye(P, dtype=np.float32)

    Npad = NT * P
    shared = dict(w1s=w1s, w2s=w2s, gs=gsd, mus=mus, onesd=onesd, cmu=cmu,
                  b1b=b1b, b2b=b2b, iota=iota, iotac=iotac, ones1=ones1,
                  ident=ident)
    in_maps = []
    for c in range(world):
        own_nodes, ownslots, rest = perms[c]
        xp = np.zeros((Npad, IN), np.float32)
        xp[ownslots] = x[own_nodes]
        xp[CAP:CAP + len(rest)] = x[rest]
        xTi = xp.reshape(NT, P, 2, P).transpose(3, 0, 2, 1).astype(np.float16)
        i1, r1 = build_layer(c, 1)
        i2, r2 = build_layer(c, 2)
        m = dict(shared)
        m.update(xTi=xTi, isd1=i1, dstrow1=r1, isd2=i2, dstrow2=r2)
        in_maps.append(m)
    return cfg, in_maps


def assemble(cfg, outs):
    N = cfg.N
    full = np.zeros((N, KH), np.float32)
    for c in range(cfg.W):
        o = np.asarray(outs[c]["outT"])      # [KH, SHARD_CAP]
        m = cfg.idxmaps[c]
        valid = m >= 0
        full[m[valid], :] = o[:, valid].T
    return full


_CACHE = {}


def kernel(**inputs):
    world = 8
    cfg, in_maps = prep_host(world=world, **inputs)
    key = (cfg.N, cfg.W, cfg.CPB1, cfg.CPB2)
    if key not in _CACHE:
        _CACHE[key] = build_program(cfg)
    nc = _CACHE[key]

    from concourse.bass_utils import run_bass_kernel_spmd
    res = run_bass_kernel_spmd(nc, in_maps, core_ids=list(range(world)))
    return assemble(cfg, res.results)
